# revision 41
# baseline (speedup 1.0000x reference)
"""Multi-head attention (B=2, S=2048, D=1024, H=16, E=64) on 8 NeuronCores.

Sharding: core c = (batch b, head-group hg) with b = c // 4, hg = c % 4.
Each core projects q/k/v for its batch into its 4 heads, runs dense
attention for those heads over the full sequence, and computes a partial
output projection with its 256 rows of Wo.  The host sums the 4 partials
per batch and adds bo (the TP all-reduce, folded into the gather step).

v4 schedule: the ScalarE exp stream (16.8M exps/core at 1 elem/cycle/lane
is the softmax-critical engine) runs as one flat 128-step pipeline over
all 8 attention blocks - even head-pair blocks first, then odd - with
scores(i)/exp(i)/pv(i-1) streaming continuously across block boundaries.
The prologue projects only what the first block needs (kh pair 0, qh pair
0 block 0); everything else (vh, kh pair 1, remaining qh, the normalize +
output projection of finished query blocks) rides the PE's slack as
filler matmuls woven between score/PV pairs. All DRAM loads are issued
up-front on one HWDGE queue in need-order (the DMA fabric serializes
transfers in trigger order), with weights pre-packed host-side so each
loads in a single DMA.

On-chip layout (everything "T" = feature-on-partitions):
  khT       [128, 2048*2] head pair hp at cols hp*S; head parity on
            partitions (even head rows 0-63, odd rows 64-127)
  qhT_sc[4] [128, 1024]   per 512-query block; cols hp*512+q
  vh        [128, 16*260] per key-block tt a [vh | ones] block of 65 per
            head - the ones column makes the PV matmul emit the softmax
            denominator as psum row 64.
  attnS_sc  [65, 4*512]   bf16 PV evictions (blk = hp*2 + parity);
            row 64 = denominators
  attnT_sc  [128, 2*512]  bf16 out-proj lhsT (features of pair j on
            partitions, cols j*512+q), normalized in place
  biases are folded into the projection matmuls via a K=1 matmul against a
  ones row (weights staged host-side with the bias as row 1024).
  Score matmul pairs use K=64 row-tiles (0,0)/(64,0) via base partitions,
  so both heads' scores stream concurrently through the PE array.
"""

import numpy as np

B, S, D, H, E = 2, 2048, 1024, 16, 64
HG = 4            # heads per core
N_CORES = 8
EL = E + 1        # 65: head block width in vh (values + ones column)
DT = D // 128     # 8 contraction tiles
SC = S // 512     # 4 query chunks of 512

_NC = None        # cached compiled Bass module

# e4[k, j*128+m] = (k == 2j + m//64): broadcasts recip row (head) k of a
# 512-query block to out partition m of the pair-j normalize tile.
_E4 = np.zeros((4, 256), np.float32)
for _j in range(2):
    for _m in range(128):
        _E4[2 * _j + _m // 64, _j * 128 + _m] = 1.0
_ONES = np.ones((1, 512), np.float32)


def _build():
    import concourse.bass as bass
    import concourse.mybir as mybir
    import concourse.tile as tile
    from concourse import bacc
    from collections import deque

    FP = mybir.dt.float32
    FPR = mybir.dt.float32r
    BF = mybir.dt.bfloat16
    EXP = mybir.ActivationFunctionType.Exp

    nc = bacc.Bacc("TRN2", target_bir_lowering=False, debug=False, num_devices=1)

    xq = nc.dram_tensor("xq", [D, S], BF, kind="ExternalInput").ap()
    xk = nc.dram_tensor("xk", [D, S], BF, kind="ExternalInput").ap()
    xv = nc.dram_tensor("xv", [D, S], BF, kind="ExternalInput").ap()
    # weights pre-packed host-side as [128, DT*cols]: col block dt holds
    # rows dt*128..dt*128+127 of the logical [D, cols] weight; biases ride
    # as separate [1, cols] tensors
    wq = nc.dram_tensor("wq", [128, DT * HG * E], BF, kind="ExternalInput").ap()
    wk = nc.dram_tensor("wk", [128, DT * HG * E], BF, kind="ExternalInput").ap()
    wv = nc.dram_tensor("wv", [128, DT * HG * EL], BF, kind="ExternalInput").ap()
    wqb_d = nc.dram_tensor("wqb", [1, HG * E], BF, kind="ExternalInput").ap()
    wkb_d = nc.dram_tensor("wkb", [1, HG * E], BF, kind="ExternalInput").ap()
    wvb_d = nc.dram_tensor("wvb", [1, HG * EL], BF, kind="ExternalInput").ap()
    wo = nc.dram_tensor("wo", [HG * E, D], BF, kind="ExternalInput").ap()
    e4_d = nc.dram_tensor("e4", [4, 256], BF, kind="ExternalInput").ap()
    ones_d = nc.dram_tensor("ones", [1, 512], BF, kind="ExternalInput").ap()
    out = nc.dram_tensor("out_partial", [S, D], BF, kind="ExternalOutput").ap()

    with tile.TileContext(nc) as tc:
        with (
            tc.tile_pool(name="consts", bufs=1) as cpool,
            tc.tile_pool(name="resident", bufs=1) as rpool,
            tc.tile_pool(name="xin", bufs=8) as xpool,
            tc.tile_pool(name="xq_thin", bufs=32) as xqpool,
            tc.tile_pool(name="xv_half", bufs=12) as xvpool,
            tc.tile_pool(name="exp", bufs=3) as epool,
            tc.tile_pool(name="outev", bufs=4) as opool,
        ):
            # DMA ordering: the aggregate DMA fabric serializes transfers in
            # trigger order, so emit loads in need-order. gpsimd (SWDGE)
            # queue: wk first (unblocks kh), then small consts, then wq/wo.
            # HWDGE (sync) queue: xk, xq block 0, wv, xv - emitted up front,
            # decoupled from the compute emission below.
            ones = cpool.tile([1, 512], BF, tag="ones")
            nc.gpsimd.dma_start(ones[:], ones_d[:])
            e4_sb = cpool.tile([4, 256], BF, tag="e4")
            nc.gpsimd.dma_start(e4_sb[:], e4_d[:])
            # touch the Exp table early so ACT_TABLE_LOAD overlaps phase 1a
            warm = cpool.tile([1, 512], BF, tag="warm")
            nc.scalar.activation(warm[:], ones[:], EXP, scale=1.0)
            wk_all = cpool.tile([128, DT * 256], BF, tag="wkA")
            wv_all = cpool.tile([128, DT * 260], BF, tag="wvA")
            wq_all = cpool.tile([128, DT * 256], BF, tag="wqA")
            wk_sb = [wk_all[:, dt * 256 : (dt + 1) * 256] for dt in range(DT)]
            wv_sb = [wv_all[:, dt * 260 : (dt + 1) * 260] for dt in range(DT)]
            wq_sb = [wq_all[:, dt * 256 : (dt + 1) * 256] for dt in range(DT)]
            wqb = cpool.tile([1, 256], BF, tag="wqb")
            wkb = cpool.tile([1, 256], BF, tag="wkb")
            wvb = cpool.tile([1, 260], BF, tag="wvb")
            nc.sync.dma_start(wk_all[:], wk[:])
            nc.sync.dma_start(wkb[:], wkb_d[:])
            wo_sb = [
                cpool.tile([128, D], BF, tag=f"wo{j}", name=f"wo{j}")
                for j in range(2)
            ]

            khT = rpool.tile([128, 2 * S], FPR, tag="khT")
            vh = rpool.tile([128, 16 * 260], BF, tag="vh")
            qhT_sc = [
                rpool.tile([128, 1024], FPR, tag=f"qhT{sc}", name=f"qhT{sc}")
                for sc in range(SC)
            ]
            attnS_sc = [
                rpool.tile([65, 4 * 512], BF, tag=f"atS{sc}", name=f"atS{sc}")
                for sc in range(SC)
            ]
            attnT_sc = [
                rpool.tile([128, 2 * 512], BF, tag=f"atT{sc}", name=f"atT{sc}")
                for sc in range(SC)
            ]
            dens = rpool.tile([4, SC * 512], BF, tag="dens")
            recip = rpool.tile([4, SC * 512], BF, tag="recip")

            # x row-blocks: xk stays resident into phase 2a (the pair-1 kh
            # filler matmuls reuse it); xq comes in as per-sc thin slices.
            xkt, xvt = [], []
            xq_thin = {}

            # ---- phase 1a: minimum work before the first exp -------------
            # kh pair 0 and qh (pair 0, blocks 0-1). vh is NOT here: it
            # rides attention block (0,0) as pipelined filler work so the
            # exp stream starts as soon as kh/qh land.
            with (
                tc.tile_pool(name="ps_k0", bufs=4, space="PSUM") as pk0,
                tc.tile_pool(name="ps_q0", bufs=1, space="PSUM") as pq0,
            ):
                # all phase-1a loads issued up front, in need-order
                for dt in range(DT):
                    t = xpool.tile([128, 2048], BF, tag="xk", name=f"xk{dt}")
                    nc.sync.dma_start(t[:], xk[dt * 128 : (dt + 1) * 128, :])
                    xkt.append(t)
                xq1 = []
                for dt in range(DT):
                    t = xqpool.tile([128, 512], BF, tag="xq1", name=f"xq1_{dt}")
                    nc.sync.dma_start(t[:], xq[dt * 128 : (dt + 1) * 128, 0:512])
                    xq1.append(t)
                xq_thin[0] = xq1
                nc.sync.dma_start(wq_all[:], wq[:])
                nc.sync.dma_start(wqb[:], wqb_d[:])
                nc.sync.dma_start(wv_all[:], wv[:])
                nc.sync.dma_start(wvb[:], wvb_d[:])
                # xv in [128, 1024] halves so vh(tt) streams behind the
                # transfers instead of waiting for the full 4 MiB
                row = []
                for dt in range(DT):
                    t = xvpool.tile([128, 1024], BF, tag="xv", name=f"xv0_{dt}")
                    nc.sync.dma_start(
                        t[:], xv[dt * 128 : (dt + 1) * 128, 0:1024]
                    )
                    row.append(t)
                xvt.append(row)
                row = []
                for dt in range(DT):
                    t = xvpool.tile([128, 1024], BF, tag="xv", name=f"xv1_{dt}")
                    nc.sync.dma_start(
                        t[:], xv[dt * 128 : (dt + 1) * 128, 1024:2048]
                    )
                    row.append(t)
                xvt.append(row)
                xq2 = []
                for dt in range(DT):
                    t = xqpool.tile([128, 512], BF, tag="xq1", name=f"xq2_{dt}")
                    nc.sync.dma_start(
                        t[:], xq[dt * 128 : (dt + 1) * 128, 512:1024]
                    )
                    xq2.append(t)
                xq_thin[1] = xq2
                for j in range(2):
                    nc.sync.dma_start(wo_sb[j][:], wo[j * 128 : (j + 1) * 128, :])
                for sc in (2, 3):
                    row = []
                    for d in range(DT):
                        tq = xqpool.tile(
                            [128, 512], BF, tag="xq1", name=f"xq_{sc}_{d}"
                        )
                        nc.sync.dma_start(
                            tq[:],
                            xq[d * 128 : (d + 1) * 128, sc * 512 : (sc + 1) * 512],
                        )
                        row.append(tq)
                    xq_thin[sc] = row

                pss = {}
                for sc in range(SC):
                    pss[sc] = pk0.tile([128, 512], FP, tag="pk", name=f"pk_{sc}")
                for dt in range(DT):
                    for sc in range(SC):
                        nc.tensor.matmul(
                            pss[sc][:],
                            wk_sb[dt][:, 0:128],
                            xkt[dt][:, sc * 512 : (sc + 1) * 512],
                            start=(dt == 0),
                            stop=False,
                        )
                for sc in range(SC):
                    nc.tensor.matmul(
                        pss[sc][:],
                        wkb[0:1, 0:128],
                        ones[0:1, :],
                        start=False,
                        stop=True,
                    )
                    nc.vector.tensor_copy(
                        khT[:, sc * 512 : (sc + 1) * 512], pss[sc][:]
                    )
                # qh (pair 0, query block 0)
                psq = pq0.tile([128, 512], FP, tag="pq0", name="pq0_0")
                for dt in range(DT):
                    nc.tensor.matmul(
                        psq[:],
                        wq_sb[dt][:, 0:128],
                        xq1[dt][:],
                        start=(dt == 0),
                        stop=False,
                    )
                nc.tensor.matmul(
                    psq[:], wqb[0:1, 0:128], ones[0:1, :], start=False, stop=True
                )
                nc.vector.tensor_copy(qhT_sc[0][:, 0:512], psq[:])

            # ---- phase 2: attention with pipelined projections -----------
            fillers = deque()

            def emit_some(n=1):
                for _ in range(n):
                    if fillers:
                        fillers.popleft()()

            with (
                tc.tile_pool(name="ps_sc", bufs=2, space="PSUM") as psc,
                tc.tile_pool(name="ps_pv", bufs=1, space="PSUM") as ppv,
                tc.tile_pool(name="ps_mix", bufs=2, space="PSUM") as pmix,
            ):
                # --- filler generators (each unit ~0.2-0.9us of PE) ---
                def proj_burst(kind, j, sc):
                    """kh pair-1 (kind='k') or qh (kind='q') projection of
                    one 512-wide block as 5 filler units sharing one pmix
                    psum: 4x(2 dt matmuls) + (bias + evict). qh bursts load
                    their sc's thin xq slices on first use (shared by j0/j1)."""
                    st = {}
                    if kind == "k":
                        w_sb, w_b = wk_sb, wkb
                        xsl = lambda dt: xkt[dt][:, sc * 512 : (sc + 1) * 512]
                    else:
                        w_sb, w_b = wq_sb, wqb

                        def xsl(dt):
                            return xq_thin[sc][dt][:]

                    def mms(d0):
                        def f():
                            if d0 == 0:
                                st["ps"] = pmix.tile([128, 512], FP, tag="mx", name="mxp")
                            for dt in (d0, d0 + 1):
                                nc.tensor.matmul(
                                    st["ps"][:],
                                    w_sb[dt][:, j * 128 : (j + 1) * 128],
                                    xsl(dt),
                                    start=(dt == 0),
                                    stop=False,
                                )

                        return f

                    def fin():
                        nc.tensor.matmul(
                            st["ps"][:],
                            w_b[0:1, j * 128 : (j + 1) * 128],
                            ones[0:1, :],
                            start=False,
                            stop=True,
                        )
                        if kind == "k":
                            dst = khT[:, j * S + sc * 512 : j * S + (sc + 1) * 512]
                        else:
                            dst = qhT_sc[sc][:, j * 512 : (j + 1) * 512]
                        nc.vector.tensor_copy(dst, st["ps"][:])

                    return [mms(0), mms(2), mms(4), mms(6), fin]

                def rb_filler(sc, j):
                    def f():
                        rb = pmix.tile([128, 512], FP, tag="mx")
                        nc.tensor.matmul(
                            rb[:],
                            e4_sb[:, j * 128 : (j + 1) * 128],
                            recip[0:4, sc * 512 : (sc + 1) * 512],
                            start=True,
                            stop=True,
                        )
                        sl = attnT_sc[sc][:, j * 512 : (j + 1) * 512]
                        nc.vector.tensor_mul(sl, sl, rb[:])

                    return f

                def op_filler(sc, stq, nh):
                    def f():
                        ps = pmix.tile([128, 512], FP, tag="mx")
                        for j in range(2):
                            nc.tensor.matmul(
                                ps[:],
                                attnT_sc[sc][:, j * 512 + stq * 128 : j * 512 + (stq + 1) * 128],
                                wo_sb[j][:, nh * 512 : (nh + 1) * 512],
                                start=(j == 0),
                                stop=(j == 1),
                            )
                        ot = opool.tile([128, 512], BF, tag="outev")
                        nc.vector.tensor_copy(ot[:], ps[:])
                        nc.sync.dma_start(
                            out[
                                (sc * 4 + stq) * 128 : (sc * 4 + stq + 1) * 128,
                                nh * 512 : (nh + 1) * 512,
                            ],
                            ot[:],
                        )

                    return f

                def vh_unit(tt):
                    """project v for key block tt into vh (ridden by the
                    first attention block as pipelined filler work)."""
                    def f():
                        ps = pmix.tile([128, 260], FP, tag="mx", name="mxv")
                        for dt in range(DT):
                            nc.tensor.matmul(
                                ps[:],
                                xvt[tt // 8][dt][:, (tt % 8) * 128 : (tt % 8 + 1) * 128],
                                wv_sb[dt][:],
                                start=(dt == 0),
                                stop=False,
                            )
                        nc.tensor.matmul(
                            ps[:], ones[0:1, 0:128], wvb[0:1, :], start=False, stop=True
                        )
                        nc.vector.tensor_copy(vh[:, tt * 260 : (tt + 1) * 260], ps[:])

                    return f

                # qh (pair 0, block 1) burst units: woven into the tail of
                # block (0,0) as second fillers (its xq slices land early)
                def q01_units():
                    stq1 = {}

                    def u(d0, n, fin=False):
                        def f():
                            if d0 == 0:
                                stq1["ps"] = pmix.tile(
                                    [128, 512], FP, tag="mx", name="pq0_1"
                                )
                            for dt in range(d0, d0 + n):
                                nc.tensor.matmul(
                                    stq1["ps"][:],
                                    wq_sb[dt][:, 0:128],
                                    xq2[dt][:],
                                    start=(dt == 0),
                                    stop=False,
                                )
                            if fin:
                                nc.tensor.matmul(
                                    stq1["ps"][:],
                                    wqb[0:1, 0:128],
                                    ones[0:1, :],
                                    start=False,
                                    stop=True,
                                )
                                nc.vector.tensor_copy(
                                    qhT_sc[1][:, 0:512], stq1["ps"][:]
                                )

                        return f

                    return [u(0, 3), u(3, 3), u(6, 2, fin=True)]

                # ---- flat 128-step pipeline over all 8 attention blocks ---
                # blocks in order: (0,sc0..3) then (1,sc0..3). scores(i) and
                # pv(i-1) stream continuously across block boundaries; PV
                # psum evictions, denominators and the normalize/out-proj
                # fillers are emitted as boundary extras.
                BLKS = [(0, sc) for sc in range(SC)] + [(1, sc) for sc in range(SC)]
                pvt_by_block = {}
                exq = {}

                def scores(b, tt):
                    hp, sc = BLKS[b]
                    ps = psc.tile([128, 1024], FP, tag="sc")
                    for hh in range(2):
                        nc.tensor.matmul(
                            ps[:, hh * 512 : (hh + 1) * 512],
                            khT[hh * 64 : (hh + 1) * 64, hp * S + tt * 128 : hp * S + (tt + 1) * 128],
                            qhT_sc[sc][hh * 64 : (hh + 1) * 64, hp * 512 : (hp + 1) * 512],
                            start=True,
                            stop=True,
                        )
                    ex = epool.tile([128, 1024], BF, tag="exp")
                    nc.scalar.activation(ex[:], ps[:], EXP, scale=0.125)
                    exq[b, tt] = ex

                def pv(b, tt):
                    hp, _ = BLKS[b]
                    ex = exq.pop((b, tt))
                    for hh in range(2):
                        h = 2 * hp + hh
                        nc.tensor.matmul(
                            pvt_by_block[b][hh][:],
                            vh[:, tt * 260 + h * EL : tt * 260 + h * EL + EL],
                            ex[:, hh * 512 : (hh + 1) * 512],
                            start=(tt == 0),
                            stop=(tt == 15),
                        )

                def evict_block(b):
                    hp, sc = BLKS[b]
                    pvt = pvt_by_block.pop(b)
                    for hh in range(2):
                        blk = hp * 2 + hh
                        nc.vector.tensor_copy(
                            attnS_sc[sc][:, blk * 512 : (blk + 1) * 512],
                            pvt[hh][:],
                        )
                    nc.vector.tensor_copy(
                        attnT_sc[sc][0:64, hp * 512 : (hp + 1) * 512],
                        attnS_sc[sc][0:64, (hp * 2) * 512 : (hp * 2 + 1) * 512],
                    )
                    nc.sync.dma_start(
                        attnT_sc[sc][64:128, hp * 512 : (hp + 1) * 512],
                        attnS_sc[sc][0:64, (hp * 2 + 1) * 512 : (hp * 2 + 2) * 512],
                    )
                    if hp == 1:
                        nc.sync.dma_start(
                            dens[0:4, sc * 512 : (sc + 1) * 512],
                            attnS_sc[sc][64:65, 0:2048],
                        )
                        with nc.allow_low_precision(reason="bf16 recip/matmul"):
                            nc.vector.reciprocal(
                                recip[0:4, sc * 512 : (sc + 1) * 512],
                                dens[0:4, sc * 512 : (sc + 1) * 512],
                            )
                        for j in range(2):
                            fillers.append(rb_filler(sc, j))
                        for stq in range(4):
                            for nh in range(2):
                                fillers.append(op_filler(sc, stq, nh))

                # filler plan for steps 16+: remaining projections in
                # need-order; rb/op fillers join the deque at boundaries
                fillers.extend(proj_burst("q", 0, 2))
                fillers.extend(proj_burst("k", 1, 0))
                fillers.extend(proj_burst("q", 1, 0))
                fillers.extend(proj_burst("k", 1, 1))
                fillers.extend(proj_burst("q", 0, 3))
                fillers.extend(proj_burst("q", 1, 1))
                fillers.extend(proj_burst("k", 1, 2))
                fillers.extend(proj_burst("q", 1, 2))
                fillers.extend(proj_burst("k", 1, 3))
                fillers.extend(proj_burst("q", 1, 3))

                vh_units = [vh_unit(tt) for tt in range(16)]
                q01 = q01_units()

                for i in range(128):
                    b, tt = i // 16, i % 16
                    if tt == 0:
                        pvt_by_block[b] = {
                            hh: ppv.tile(
                                [EL, 512], FP, tag=f"pv{hh}", name=f"pv{hh}_{b}"
                            )
                            for hh in range(2)
                        }
                    scores(b, tt)
                    if i < 16:
                        # block (0,0): vh projection rides as filler, pv
                        # chases it with a 2-step lag
                        if i >= 2:
                            vh_units[i - 2]()
                            pv(0, i - 2)
                        if i >= 13:
                            q01[i - 13]()
                        if i == 15:
                            vh_units[14]()
                            pv(0, 14)
                            vh_units[15]()
                            pv(0, 15)
                    elif i == 16:
                        evict_block(0)
                        emit_some(1)
                    else:
                        if tt == 1:
                            # previous block's psum eviction is still in
                            # flight on DVE: run the filler first
                            emit_some(1)
                            pv(*divmod(i - 1, 16))
                        else:
                            pv(*divmod(i - 1, 16))
                            if tt == 0:
                                evict_block(b - 1)
                            emit_some(1)
                pv(7, 15)
                evict_block(7)
                while fillers:
                    fillers.popleft()()

    nc.compile()
    return nc


def _get_nc():
    global _NC
    if _NC is None:
        _NC = _build()
    return _NC


def _in_maps(q, k, v, Wq, bq, Wk, bk, Wv, bv, Wo, bo):
    import ml_dtypes
    f32 = np.float32
    bf16 = ml_dtypes.bfloat16
    maps = []
    for c in range(N_CORES):
        b, hg = c // HG, c % HG
        hs = slice(hg * HG, (hg + 1) * HG)  # this core's 4 heads

        def pack(w):
            # [D, C] -> [128, DT*C]: col block dt = rows dt*128..dt*128+127
            Dd, C = w.shape
            return np.transpose(w.reshape(DT, 128, C), (1, 0, 2)).reshape(128, DT * C)

        wq_h = pack(np.ascontiguousarray(
            np.transpose(Wq[hs], (1, 0, 2)).reshape(D, HG * E)))
        wqb_h = bq[hs].reshape(1, -1)
        wk_h = pack(np.ascontiguousarray(
            np.transpose(Wk[hs], (1, 0, 2)).reshape(D, HG * E)))
        wkb_h = bk[hs].reshape(1, -1)
        wv_full = np.zeros((D, HG * EL), f32)
        wvb_h = np.zeros((1, HG * EL), f32)
        for hl in range(HG):
            wv_full[:, hl * EL : hl * EL + E] = Wv[hg * HG + hl]
            wvb_h[0, hl * EL : hl * EL + E] = bv[hg * HG + hl]
            wvb_h[0, hl * EL + E] = 1.0  # generates the ones column of vh
        wv_h = pack(wv_full)
        maps.append(
            {
                "xq": np.ascontiguousarray(q[b].T).astype(bf16),
                "xk": np.ascontiguousarray(k[b].T).astype(bf16),
                "xv": np.ascontiguousarray(v[b].T).astype(bf16),
                "wq": wq_h.astype(bf16),
                "wk": wk_h.astype(bf16),
                "wv": wv_h.astype(bf16),
                "wqb": np.asarray(wqb_h, f32).astype(bf16),
                "wkb": np.asarray(wkb_h, f32).astype(bf16),
                "wvb": np.asarray(wvb_h, f32).astype(bf16),
                "wo": np.ascontiguousarray(
                    Wo[hg * HG * E : (hg + 1) * HG * E, :]
                ).astype(bf16),
                "e4": _E4.astype(bf16),
                "ones": _ONES.astype(bf16),
            }
        )
    return maps


def _run(inputs, trace=False):
    from concourse.bass_utils import run_bass_kernel_spmd

    nc = _get_nc()
    maps = _in_maps(**inputs)
    res = run_bass_kernel_spmd(nc, maps, list(range(N_CORES)), trace=trace)
    bo = np.asarray(inputs["bo"], np.float32)
    out = np.zeros((B, S, D), np.float32)
    for b in range(B):
        acc = np.zeros((S, D), np.float32)
        for hg in range(HG):
            acc += np.asarray(res.results[b * HG + hg]["out_partial"], np.float32)
        out[b] = acc + bo[None, :]
    return out, res.exec_time_ns


def kernel(**inputs):
    out, _ = _run(inputs, trace=False)
    return out


def kernel_traced(**inputs):
    return _run(inputs, trace=True)
